# revision 22
# baseline (speedup 1.0000x reference)
"""AceStep GQA attention block on 8 TRN2 NeuronCores.

Sharding: tensor-parallel over heads (TP=2, kv heads stay grouped with
their q heads) x data-parallel over batch (DP=4).  Core i handles batch
b = i // 2 and head group g = i % 2 (q heads 8g..8g+7, kv heads 2g,2g+1).
Each core computes a partial output projection (its head group's slice of
Wo rows); the host sums the two partials per batch.

Device-side dataflow per core (all matmuls f32r = full-rate fp32):
  pass p in {0,1}:  (kv head p, q heads 4p..4p+3)
    proj:  xT tiles (stationary) x Wq/Wk/Wv slices -> Q/K/V token-major,
           per-head RMSNorm via ACT square+accum, rsqrt; RoPE fused with the
           norm scale via scalar_tensor_tensor (cos/sin tables carry the
           norm weights, folded on host); PE-transpose Q,K to [d, t].
    attn:  ST[sk,sq] = KT_tile.T @ QT chunk; E = exp(SCALE*ST) on ACT;
           denominator = ones.T @ E (PE, accumulated over sk tiles);
           OUT_T[d,sq] = V_tile.T @ E accumulated over sk tiles;
           A = OUT_T * bcast(1/denom)  (bcast via K=1 matmul).
  final: out[t,:] += A_h[:,t].T @ Wo_h rows, accumulated over 8 heads.
"""

import sys

if "/opt/trn_rl_repo" not in sys.path:
    sys.path.insert(0, "/opt/trn_rl_repo")

from contextlib import ExitStack

import numpy as np
import ml_dtypes

import concourse.bass as bass
import concourse.mybir as mybir
import concourse.tile as tile
from concourse.bass import ts, ds
from concourse.masks import make_identity
from concourse.vector_clock import ScopedClock, VectorClock
from concourse.bass_utils import run_bass_kernel_spmd

B, S, HID = 4, 2048, 2048
H, KV, D = 16, 4, 128
EPS = 1e-6
SCALE = float(D) ** -0.5
NCORES = 8
TP = 2
QH = H // TP            # 8 q heads per core
KVH = KV // TP          # 2 kv heads per core = passes
QHP = QH // KVH         # 4 q heads per pass
NT = S // 128           # 16 token tiles
NHID = HID // 128       # 16 hid tiles
CH = 512                # sq chunk width
NCH = S // CH           # 4 chunks
F32 = mybir.dt.float32
F32R = mybir.dt.float32r
BF16 = mybir.dt.bfloat16
CH2 = 1024              # paired sq chunk width (2 PSUM banks)
MULT = mybir.AluOpType.mult
AF = mybir.ActivationFunctionType


def _patched_drain_and_barrier(self, tick_clock, wait_clock):
    # Walrus CoreV3 rejects >1-2 sem waits on a CTRL (Drain) instruction.
    # Split the final global-clock wait into one single-wait drain per proc.
    gc = tick_clock.global_clock
    n = len(gc)
    for p in range(n):
        t = gc[p]
        if t > 0:
            vec = [0] * n
            vec[p] = t
            d = self.nc.sync.drain()
            wait_clock.add_sem_waits(d.ins, ScopedClock({None: VectorClock(vec)}))
    self.nc.sync.drain()
    self.nc.all_engine_barrier()
    assert self.sems is not None
    popped = self.nc._tile_sem_poison_stack.pop()
    assert popped is self._sem_poison
    self.nc.clear_and_free_semaphores(list(self.sems.allocated().values()))
    self.nc.all_engine_barrier()


tile.TileContext._drain_and_barrier = _patched_drain_and_barrier

def _max_waits(inst):
    # Walrus CoreV2/V3 setupSyncWait takes a single wait per TPB instruction;
    # EventSemaphore can hold two.
    if isinstance(inst, mybir.InstEventSemaphore):
        return 2
    return 1


def _legalize_waits(nc):
    """Walrus CoreV3 rejects instructions carrying too many sync waits.
    Spill the excess onto no-op carrier instructions inserted just before,
    on the same engine stream."""
    n_new = 0
    for f in nc.m.functions:
        for bb in f.blocks:
            insts = bb.instructions
            out = []
            changed = False
            for inst in insts:
                si = getattr(inst, "sync_info", None)
                waits = list(si.on_wait) if (si and si.on_wait) else []
                mw = _max_waits(inst)
                if len(waits) > mw:
                    spill, keep = waits[:-mw], waits[-mw:]
                    for i in range(0, len(spill)):
                        nop = mybir.InstNoOp(
                            name=f"waitspill-{n_new}",
                            engine=inst.engine,
                            sync_info=mybir.SyncInfo(
                                on_wait=spill[i : i + 1], on_update=[]
                            ),
                            bass_nofuse=True,
                        )
                        n_new += 1
                        out.append(nop)
                    si.on_wait = keep
                    changed = True
                out.append(inst)
            if changed:
                bb.instructions = out
    return n_new


def _emit(nc, tc, io):
    xT, wq, wkv, cwq, swq, cwk, swk, wo, ones_d, out = io

    xT = xT.rearrange("(j p) t -> p j t", p=128)       # [128, NHID, S]
    wq = wq.rearrange("(j p) n -> p j n", p=128)       # [128, NHID, QH*D]
    wkv = wkv.rearrange("(j p) a n -> p j a n", p=128)  # [128, NHID, KVH, 256]
    wo_r = wo.rearrange("(h p) n -> p h n", p=128)     # [128, QH, HID]

    with ExitStack() as top:
        const = top.enter_context(tc.tile_pool(name="const", bufs=1))
        ident = const.tile([128, 128], BF16)
        make_identity(nc, ident)
        ones_col = const.tile([128, 1], BF16)
        nc.sync.dma_start(out=ones_col, in_=ones_d[:, 0:1])
        ones_row_b = const.tile([1, 128], BF16)
        nc.sync.dma_start(out=ones_row_b, in_=ones_d[0:1, :])
        ones_row = const.tile([1, 128], F32R)
        eps_t = const.tile([128, 1], F32)
        nc.vector.memset(eps_t, EPS)
        # pin the ACT table set to natural_log_exp_and_others (has exp, ln,
        # square, copy) so no table switches happen mid-kernel
        dummy = const.tile([128, 1], F32)
        nc.scalar.activation(dummy, eps_t, AF.Ln)
        nc.scalar.copy(ones_row, ones_row_b)

        qa_pool = top.enter_context(tc.tile_pool(name="qa", bufs=1))
        QA = qa_pool.tile([128, QH, S], BF16)           # QT, later A, [d, h, t]
        kt_pool = top.enter_context(tc.tile_pool(name="kt", bufs=1))
        KT = kt_pool.tile([128, S], BF16)               # per-pass KT [d, t]
        v_pool = top.enter_context(tc.tile_pool(name="v", bufs=1))
        VT = v_pool.tile([128, NT, D], BF16)            # per-pass V [t%128, tt, d]

        epool = top.enter_context(tc.tile_pool(name="e", bufs=6))
        small = top.enter_context(tc.tile_pool(name="small", bufs=12))

        wo_sb = None

        for p in range(KVH):
            # ---------------- projection phase (pass p) ----------------
            with ExitStack() as ph:
                wq_pool = ph.enter_context(tc.tile_pool(name="wq", bufs=1))
                wq_sb = wq_pool.tile([128, NHID, QHP * D], BF16)
                wkv_sb = wq_pool.tile([128, NHID, 256], BF16)
                for j in range(NHID):
                    nc.sync.dma_start(
                        out=wq_sb[:, j, :], in_=wq[:, j, ds(p * QHP * D, QHP * D)]
                    )
                    nc.sync.dma_start(out=wkv_sb[:, j, :], in_=wkv[:, j, p, :])

                xpool = ph.enter_context(tc.tile_pool(name="x", bufs=3))
                rpool = ph.enter_context(tc.tile_pool(name="rope", bufs=2))
                spool = ph.enter_context(tc.tile_pool(name="scr", bufs=4))
                psq = ph.enter_context(tc.tile_pool(name="psq", bufs=2, space="PSUM"))
                pskv = ph.enter_context(tc.tile_pool(name="pskv", bufs=2, space="PSUM"))
                pst_pool = ph.enter_context(
                    tc.tile_pool(name="pst", bufs=4, space="PSUM")
                )

                for tt in range(NT):
                    xx = xpool.tile([128, NHID, 128], BF16, tag="xx")
                    for j in range(NHID):
                        nc.sync.dma_start(out=xx[:, j, :], in_=xT[:, j, ts(tt, 128)])
                    cwq_t = rpool.tile([128, 128], F32, tag="cwq")
                    swq_t = rpool.tile([128, 128], F32, tag="swq")
                    cwk_t = rpool.tile([128, 128], F32, tag="cwk")
                    swk_t = rpool.tile([128, 128], F32, tag="swk")
                    nc.sync.dma_start(out=cwq_t, in_=cwq[ts(tt, 128), :])
                    nc.sync.dma_start(out=swq_t, in_=swq[ts(tt, 128), :])
                    nc.sync.dma_start(out=cwk_t, in_=cwk[ts(tt, 128), :])
                    nc.sync.dma_start(out=swk_t, in_=swk[ts(tt, 128), :])

                    psQ = psq.tile([128, QHP * D], F32)
                    psKV = pskv.tile([128, 256], F32)
                    for j in range(NHID):
                        nc.tensor.matmul(
                            psQ,
                            xx[:, j, :],
                            wq_sb[:, j, :],
                            start=(j == 0),
                            stop=(j == NHID - 1),
                        )
                    for j in range(NHID):
                        nc.tensor.matmul(
                            psKV,
                            xx[:, j, :],
                            wkv_sb[:, j, :],
                            start=(j == 0),
                            stop=(j == NHID - 1),
                        )

                    def norm_rope_transpose(src, cw_t, sw_t, dst):
                        scratch = spool.tile([128, 128], F32, tag="scr")
                        ssq = small.tile([128, 1], F32, tag="ssq")
                        s1 = small.tile([128, 1], F32, tag="s1")
                        r = small.tile([128, 1], F32, tag="r")
                        nc.scalar.activation(scratch, src, AF.Square, accum_out=ssq)
                        nc.scalar.activation(
                            s1, ssq, AF.Ln, bias=eps_t, scale=1.0 / D
                        )
                        nc.scalar.activation(r, s1, AF.Exp, scale=-0.5)
                        m1 = spool.tile([128, 128], F32, tag="m1")
                        m2 = spool.tile([128, 128], F32, tag="m2")
                        qr = spool.tile([128, 128], BF16, tag="qr")
                        nc.vector.scalar_tensor_tensor(
                            out=m1, in0=src, scalar=r, in1=cw_t, op0=MULT, op1=MULT
                        )
                        nc.vector.scalar_tensor_tensor(
                            out=m2[:, 0:64],
                            in0=src[:, 64:128],
                            scalar=r,
                            in1=sw_t[:, 0:64],
                            op0=MULT,
                            op1=MULT,
                        )
                        nc.vector.scalar_tensor_tensor(
                            out=m2[:, 64:128],
                            in0=src[:, 0:64],
                            scalar=r,
                            in1=sw_t[:, 64:128],
                            op0=MULT,
                            op1=MULT,
                        )
                        nc.vector.tensor_add(qr, m1, m2)
                        psT = pst_pool.tile([128, 128], BF16)
                        nc.tensor.transpose(psT, qr, ident)
                        nc.scalar.copy(dst, psT)

                    for jh in range(QHP):
                        hl = p * QHP + jh
                        norm_rope_transpose(
                            psQ[:, ts(jh, 128)], cwq_t, swq_t, QA[:, hl, ts(tt, 128)]
                        )
                    norm_rope_transpose(
                        psKV[:, 0:128], cwk_t, swk_t, KT[:, ts(tt, 128)]
                    )
                    nc.scalar.copy(VT[:, tt, :], psKV[:, 128:256])

            # load Wo after the last projection phase frees its pools
            if p == KVH - 1:
                wo_pool = top.enter_context(tc.tile_pool(name="wo", bufs=1))
                wo_sb = wo_pool.tile([128, QH, HID], BF16)
                for h in range(QH):
                    nc.sync.dma_start(out=wo_sb[:, h, :], in_=wo_r[:, h, :])

            # ---------------- attention phase (pass p) ----------------
            # Processed in sq chunk-pairs of 1024: scores fill a 2-bank PSUM
            # tile, one wide exp per sk tile (amortizes ACT per-op overhead),
            # denominator 2-way column-tiled on the PE (concurrent groups).
            with ExitStack() as ph:
                pss = ph.enter_context(tc.tile_pool(name="pss", bufs=2, space="PSUM"))
                pso = ph.enter_context(tc.tile_pool(name="pso", bufs=2, space="PSUM"))
                psd = ph.enter_context(tc.tile_pool(name="psd", bufs=2, space="PSUM"))

                for jh in range(QHP):
                    hl = p * QHP + jh
                    for cp in range(S // CH2):
                        etiles = [None] * NT

                        def scores(i):
                            psS = pss.tile([128, CH2], F32, tag="s")
                            for h2 in range(2):
                                nc.tensor.matmul(
                                    psS[:, ds(h2 * CH, CH)],
                                    KT[:, ts(i, 128)],
                                    QA[:, hl, ds(cp * CH2 + h2 * CH, CH)],
                                    start=True,
                                    stop=True,
                                )
                            e = epool.tile([128, CH2], BF16, tag="e")
                            nc.scalar.activation(e, psS, AF.Exp, scale=SCALE)
                            etiles[i] = e

                        psD = [psd.tile([128, CH], F32, tag="d", name=f"psD{_h}") for _h in range(2)]
                        psO = [pso.tile([128, CH], F32, tag="o", name=f"psO{_h}") for _h in range(2)]
                        scores(0)
                        scores(1)
                        for i in range(NT):
                            e = etiles[i]
                            g = i % 2
                            for h2 in range(2):
                                eh = e[:, ds(h2 * CH, CH)]
                                nc.tensor.matmul(
                                    psD[h2][32 * g : 32 * g + 1, :],
                                    ones_col,
                                    eh,
                                    start=(i < 2),
                                    stop=(i >= NT - 2),
                                    tile_position=(0, 32 * g),
                                )
                                nc.tensor.matmul(
                                    psO[h2],
                                    VT[:, i, :],
                                    eh,
                                    start=(i == 0),
                                    stop=(i == NT - 1),
                                )
                            if i + 2 < NT:
                                scores(i + 2)
                        for h2 in range(2):
                            c = cp * 2 + h2
                            t1 = small.tile([1, CH], F32, tag="cmb")
                            nc.vector.tensor_copy(t1, psD[h2][32:33, :])
                            rdp = small.tile([1, CH], F32, tag="rdp")
                            nc.vector.tensor_add(rdp, psD[h2][0:1, :], t1)
                            rd = small.tile([1, CH], F32R, tag="rd")
                            with nc.allow_low_precision(reason="f32r bcast rhs"):
                                nc.vector.reciprocal(rd, rdp)
                            psB = pss.tile([128, CH], F32, tag="s")
                            nc.tensor.matmul(
                                psB, ones_row, rd, start=True, stop=True
                            )
                            bc = epool.tile([128, CH], F32, tag="e")
                            nc.vector.tensor_copy(bc, psB)
                            nc.vector.tensor_mul(
                                QA[:, hl, ds(c * CH, CH)], psO[h2], bc
                            )

        # ---------------- output projection ----------------
        with ExitStack() as ph:
            psc = ph.enter_context(tc.tile_pool(name="psc", bufs=6, space="PSUM"))
            opool = ph.enter_context(tc.tile_pool(name="osb", bufs=6))
            for tt in range(NT):
                for nch in range(NCH):
                    psC = psc.tile([128, CH], F32)
                    for h in range(QH):
                        nc.tensor.matmul(
                            psC,
                            QA[:, h, ts(tt, 128)],
                            wo_sb[:, h, ds(nch * CH, CH)],
                            start=(h == 0),
                            stop=(h == QH - 1),
                        )
                    osb = opool.tile([128, CH], F32, tag="osb")
                    if nch % 2 == 0:
                        nc.scalar.copy(osb, psC)
                    else:
                        nc.vector.tensor_copy(osb, psC)
                    nc.sync.dma_start(
                        out=out[ts(tt, 128), ds(nch * CH, CH)], in_=osb
                    )


_PROGRAM = None


def _build_program(legalize=True, bodies=1):
    global _PROGRAM
    if _PROGRAM is not None and legalize and bodies == 1:
        return _PROGRAM
    nc = bass.Bass("TRN2", target_bir_lowering=False, debug=False, num_devices=NCORES)
    xT = nc.dram_tensor("xT", [HID, S], BF16, kind="ExternalInput").ap()
    wq = nc.dram_tensor("wq", [HID, QH * D], BF16, kind="ExternalInput").ap()
    wkv = nc.dram_tensor("wkv", [HID, KVH, 256], BF16, kind="ExternalInput").ap()
    cwq = nc.dram_tensor("cwq", [S, D], F32, kind="ExternalInput").ap()
    swq = nc.dram_tensor("swq", [S, D], F32, kind="ExternalInput").ap()
    cwk = nc.dram_tensor("cwk", [S, D], F32, kind="ExternalInput").ap()
    swk = nc.dram_tensor("swk", [S, D], F32, kind="ExternalInput").ap()
    wo = nc.dram_tensor("wo", [QH * D, HID], BF16, kind="ExternalInput").ap()
    ones_d = nc.dram_tensor("ones", [128, 128], BF16, kind="ExternalInput").ap()
    out = nc.dram_tensor("out", [S, HID], F32, kind="ExternalOutput").ap()
    with tile.TileContext(nc) as tc:
        for _rep in range(bodies):
            _emit(nc, tc, (xT, wq, wkv, cwq, swq, cwk, swk, wo, ones_d, out))
    if legalize:
        _legalize_waits(nc)
        if bodies == 1:
            _PROGRAM = nc
    return nc


def _host_prep(hidden_states, cos, sin, Wq, Wk, Wv, Wo, q_norm_w, k_norm_w):
    """Build per-core input maps."""
    f = np.float32
    cos = np.asarray(cos, f)
    sin = np.asarray(sin, f)
    qw = np.asarray(q_norm_w, f)
    kw = np.asarray(k_norm_w, f)

    def fold(w):
        cw = (cos * w[None, :]).astype(f)
        sw = np.empty_like(sin)
        half = D // 2
        sw[:, :half] = -sin[:, :half] * w[None, half:]
        sw[:, half:] = sin[:, half:] * w[None, :half]
        return np.ascontiguousarray(cw), np.ascontiguousarray(sw)

    cwq, swq = fold(qw)
    cwk, swk = fold(kw)

    Wq = np.asarray(Wq, f)
    Wk = np.asarray(Wk, f)
    Wv = np.asarray(Wv, f)
    Wo = np.asarray(Wo, f)
    hs = np.asarray(hidden_states, f)

    bf = ml_dtypes.bfloat16
    in_maps = []
    for i in range(NCORES):
        b, g = i // TP, i % TP
        xT = np.ascontiguousarray(hs[b].T).astype(bf)           # [HID, S]
        wq_g = np.ascontiguousarray(
            Wq[:, g * QH * D:(g + 1) * QH * D]
        ).astype(bf)
        wkv = np.empty((HID, KVH, 256), f)
        for p in range(KVH):
            kvh = g * KVH + p
            wkv[:, p, 0:128] = Wk[:, kvh * D:(kvh + 1) * D]
            wkv[:, p, 128:256] = Wv[:, kvh * D:(kvh + 1) * D]
        wo_g = np.ascontiguousarray(Wo[g * QH * D:(g + 1) * QH * D, :]).astype(bf)
        in_maps.append(
            {
                "xT": xT,
                "wq": wq_g,
                "wkv": wkv.astype(bf),
                "cwq": cwq,
                "swq": swq,
                "cwk": cwk,
                "swk": swk,
                "wo": wo_g,
                "ones": np.ones((128, 128), bf),
            }
        )
    return in_maps


def run_cores(in_maps, trace=False, **kwargs):
    nc = _build_program()
    return run_bass_kernel_spmd(
        nc, in_maps, core_ids=list(range(NCORES)), trace=trace, **kwargs
    )


def kernel(hidden_states, cos, sin, Wq, Wk, Wv, Wo, q_norm_w, k_norm_w):
    in_maps = _host_prep(
        hidden_states, cos, sin, Wq, Wk, Wv, Wo, q_norm_w, k_norm_w
    )
    res = run_cores(in_maps, trace=False)
    out = np.empty((B, S, HID), np.float32)
    for b in range(B):
        out[b] = res.results[b * TP]["out"]
        out[b] += res.results[b * TP + 1]["out"]
    return out
